# revision 2
# baseline (speedup 1.0000x reference)
# Trainium2 Bass kernel for AttentionBlock (conv-qkv + spatial softmax attention
# + 1x1 conv out + residual), data-parallel over batch on 8 NeuronCores.
#
# v3: four-engine balance (PE / ACT / DVE / GPS all ~equally loaded).
#   - Convs as in v2: img2col fp8 image, DoubleRow fp8 matmuls, u = Wo.Wv
#     folded, computed transposed on the PE.
#   - E = exp(S) is split by i-column: cols [0, A8) are written fp8e4 by the
#     ACT, cols [A8, IB) are written fp16 by the DVE via the int16
#     Schraudolph 2^x bit-trick (per-column scale offset cancels in the
#     per-column softmax normalize).
#   - O: the fp8 region runs as DoubleRow fp8 over jt-PAIRS (2 j-tiles
#     contracted per pass, halving PE time); the fp16 tail runs as plain
#     matmuls. uT is stored fp8 pair-interleaved [j, pair, 2, c].
#   - colsum: fp8 region split between GpSimd (~55%, it idles otherwise) and
#     DVE (1x rate on fp8); fp16 tail on DVE at 2x. Per-pair batched ops.
#   - Head: weights/xc on parallel DMA queues, 512-pixel first chunk, early
#     ACT-table / GPS-library / PE-clock warmups.

import numpy as np

try:
    import concourse.bass as bass  # noqa: F401
except ImportError:  # pragma: no cover
    import sys

    sys.path.insert(0, "/opt/trn_rl_repo")

import concourse.bass as bass  # noqa: F401
import concourse.mybir as mybir
import ml_dtypes
from concourse import bacc
from concourse import tile

B = 8
C = 128
H = W = 64
N = H * W  # 4096
NTAP = 9
IB = 1024  # attention i-block (columns per PSUM residency)
NIB = N // IB  # 4
NJT = N // 128  # 32 j-tiles per ib
NPAIR = NJT // 2  # 16 jt-pairs
NSLOT = NIB * NJT  # 128 global slots
WSCALE = 32.0  # fp8 weight pre-scale (host)
SCALE = float(C) ** -0.5
EXP_BIAS = -3.0

F32 = mybir.dt.float32
F16 = mybir.dt.float16
F8 = mybir.dt.float8e4
I16 = mybir.dt.int16
NP8 = ml_dtypes.float8_e4m3
DRm = mybir.MatmulPerfMode.DoubleRow
ADD = mybir.AluOpType.add

# E column split: [0, A8) fp8 (ACT-written, DR O-matmul), [A8, IB) fp16
# (DVE bit-trick, plain O-matmul). A8 must be even*16 for DR strides and
# divisible by 2 for the two DR column halves.
A8 = 832
E16 = IB - A8  # 192
# GPS share of the fp8 colsum columns, per ib (GPS runs ~2.4x slower than
# roofline; DVE carries the rest at 1x fp8 rate).
GS = (512, 480, 480, 480)
# fp16 2^x Schraudolph constants (see v2)
A16 = 1024.0 * 1.4426950408889634
K0 = 17156.0

_CACHE = {}


def _build_nc():
    nc = bacc.Bacc(None)

    xc_d = nc.dram_tensor("xc", [C, NTAP, N], F8, kind="ExternalInput")
    wks_d = nc.dram_tensor("wks", [C, 3, NTAP, C], F8, kind="ExternalInput")
    bqk_d = nc.dram_tensor("bqk", [C, 2], F32, kind="ExternalInput")
    xr_d = nc.dram_tensor("xr", [C, N], F16, kind="ExternalInput")
    out_d = nc.dram_tensor("out", [C, H, W], F16, kind="ExternalOutput")

    with tile.TileContext(nc) as tc:
        with tc.tile_pool(name="persist", bufs=1) as pp:
            xc = pp.tile([C, NTAP, N], F8)
            xres = pp.tile([C, N], F16)
            qb = pp.tile([C, N], F16)
            kb = pp.tile([C, N], F16)
            uT2 = pp.tile([C, NPAIR, 2, C], F8)  # [j-in-tile, pair, r, c]
            wks = pp.tile([C, 3, NTAP, C], F8)  # packed wk, wq, wu
            bqk = pp.tile([C, 2], F32)
            wk_s, wq_s, wu_s = wks[:, 0], wks[:, 1], wks[:, 2]
            bq_s, bk_s = bqk[:, 0:1], bqk[:, 1:2]
            ebias = pp.tile([C, 1], F32)
            ones = pp.tile([C, 1], F16)
            rcp = pp.tile([C, NIB, 8], F32)
            wrm = pp.tile([C, 512], F16)
            gwrm = pp.tile([C, 8], F16)

            # -------- engine warmups first (before any DMA deps) --------
            nc.vector.memset(wrm, 0.0)
            nc.vector.memset(ebias, EXP_BIAS)
            nc.vector.memset(ones, WSCALE)
            # GPS: load the tensor_tensor microcode library early
            nc.gpsimd.memset(gwrm, 0.0)
            nc.gpsimd.tensor_tensor(gwrm, gwrm, gwrm, ADD)
            # ACT: pull the exp table load into the DMA window
            awrm = pp.tile([C, 1], F16)
            nc.scalar.activation(
                awrm, ebias, mybir.ActivationFunctionType.Exp, bias=0.0, scale=1.0
            )

            # -------- input DMAs on parallel queues --------
            # sync queue: weights first, then the back half of xc, then xres.
            nc.sync.dma_start(wks, wks_d[:])
            nc.sync.dma_start(bqk, bqk_d[:])
            # scalar queue: front half of xc in consumption order; a small
            # first chunk lets the k0 conv start ~2us earlier.
            nc.scalar.dma_start(xc[:, :, 0:512], xc_d[:, :, 0:512])
            nc.scalar.dma_start(xc[:, :, 512:1024], xc_d[:, :, 512:1024])
            nc.scalar.dma_start(xc[:, :, 1024:2048], xc_d[:, :, 1024:2048])
            nc.sync.dma_start(xc[:, :, 2048:3072], xc_d[:, :, 2048:3072])
            nc.sync.dma_start(xc[:, :, 3072:4096], xc_d[:, :, 3072:4096])
            nc.sync.dma_start(xres, xr_d[:])

            cps = tc.alloc_tile_pool(name="cps", bufs=2, space="PSUM")
            # PE clock warmup while DMAs land
            wps = cps.tile([C, 512], F32, tag="conv", name="wps")
            for _ in range(6):
                nc.tensor.matmul(
                    wps[0:64, :], wrm[:, 0:64], wrm, start=True, stop=True
                )
            sps = tc.alloc_tile_pool(name="sps", bufs=2, space="PSUM")
            ops = tc.alloc_tile_pool(name="ops", bufs=1, space="PSUM")
            ep = tc.alloc_tile_pool(name="ep", bufs=3)
            ap = tc.alloc_tile_pool(name="accp", bufs=2)
            fin = tc.alloc_tile_pool(name="fin", bufs=3)
            dsp = tc.alloc_tile_pool(name="dstage", bufs=1, space="DRAM")
            rstage = dsp.tile([N], F32)

            # ---------------- conv emission helpers ----------------
            qk_ps = {}

            def qk_pair(ps, w_s, p0, pr):
                for ph in range(2):
                    nc.tensor.matmul(
                        ps[:, ph * 256 : (ph + 1) * 256],
                        w_s[:, 2 * pr : 2 * pr + 2, :],
                        xc[:, 2 * pr : 2 * pr + 2, p0 + ph * 256 : p0 + (ph + 1) * 256],
                        start=False, stop=(pr == 3 and ph == 1), perf_mode=DRm,
                    )

            def qk_group(w_s, b_s, dest, t, g):
                p0 = t * 512
                if g == 0:
                    ps = cps.tile([C, 512], F32, tag="conv", name="cps")
                    qk_ps[id(w_s), t] = ps
                    nc.tensor.matmul(
                        ps, w_s[:, 8, :], xc[:, 8, p0 : p0 + 512],
                        start=True, stop=False,
                    )
                    qk_pair(ps, w_s, p0, 0)
                    return
                ps = qk_ps[id(w_s), t]
                qk_pair(ps, w_s, p0, g)
                if g == 3:
                    del qk_ps[id(w_s), t]
                    nc.vector.tensor_scalar_add(
                        dest[:, p0 : p0 + 512], ps, b_s
                    )

            # u-conv j-tile j (transposed): stationary = image window,
            # moving = wu. Lands [128 pix, 128 ch] into a 4-jt psum batch,
            # evicted fp8 pair-interleaved into uT2.
            u_ps = {}

            def u_jt(j):
                b = j // 4
                if j % 4 == 0:
                    u_ps[b] = cps.tile([C, 512], F32, tag="conv", name="ups")
                ps = u_ps[b]
                sl = slice((j % 4) * 128, (j % 4 + 1) * 128)
                jp = j * 128
                first = j % 4 == 0
                last = j % 4 == 3
                nc.tensor.matmul(
                    ps[:, sl], xc[:, 8, jp : jp + 128], wu_s[:, 8, :],
                    start=first, stop=False,
                )
                for pr in range(4):
                    nc.tensor.matmul(
                        ps[:, sl],
                        xc[:, 2 * pr : 2 * pr + 2, jp : jp + 128],
                        wu_s[:, 2 * pr : 2 * pr + 2, :],
                        start=False, stop=(last and pr == 3), perf_mode=DRm,
                    )
                if j % 4 == 3:
                    nc.vector.tensor_copy(
                        uT2[:, b * 2 : b * 2 + 2, :, :], u_ps.pop(b)
                    )

            # ---------------- per-slot conv work lists ----------------
            work = [[] for _ in range(NSLOT)]
            pre = [[] for _ in range(NSLOT)]
            prologue = []

            for g in range(4):
                prologue.append(lambda g=g: qk_group(wk_s, bk_s, kb, 0, g))
            for t in range(2):
                for g in range(4):
                    prologue.append(lambda t=t, g=g: qk_group(wq_s, bq_s, qb, t, g))
            for j in range(4):
                prologue.append(lambda j=j: u_jt(j))
            us = 0
            for j in range(4, NJT):
                us = max(us, 1, j - 6)
                work[us].append(lambda j=j: u_jt(j))
            for T in range(1, 8):
                for g in range(4):
                    work[max(0, 4 * T - 6 + g)].append(
                        lambda T=T, g=g: qk_group(wk_s, bk_s, kb, T, g)
                    )
            for gg, s in zip(range(4), (26, 26, 27, 27)):
                work[s].append(lambda g=gg: qk_group(wq_s, bq_s, qb, 2, g))
            for gg, s in zip(range(4), (28, 28, 29, 29)):
                work[s].append(lambda g=gg: qk_group(wq_s, bq_s, qb, 3, g))
            for i in range(1, NIB - 1):
                for gg in range(8):
                    t, g = 2 * i + 2 + gg // 4, gg % 4
                    work[32 * i + 16 + gg].append(
                        lambda t=t, g=g: qk_group(wq_s, bq_s, qb, t, g)
                    )

            # ---------------- S matmul ----------------
            def s_mm(gs):
                ib, jt = gs // NJT, gs % NJT
                sp = sps.tile([C, IB], F32, tag="sp", name="sp")
                for h in range(IB // 512):
                    nc.tensor.matmul(
                        sp[:, h * 512 : (h + 1) * 512],
                        kb[:, jt * 128 : (jt + 1) * 128],
                        qb[:, ib * IB + h * 512 : ib * IB + (h + 1) * 512],
                        start=True, stop=True,
                    )
                return sp

            # ---------------- main loop ----------------
            for fn in prologue:
                fn()

            sp = s_mm(0)
            cs_pending = None
            for ib in range(NIB):
                isl = slice(ib * IB, (ib + 1) * IB)
                ob = ops.tile([C, IB], F32, tag="ob", name="ob")
                acc8 = ap.tile([C, 2, A8], F16, tag="acc8", name="acc8")
                acc16 = ap.tile([C, 2, E16], F16, tag="acc16", name="acc16")
                nc.gpsimd.memset(acc8, 0.0)
                nc.vector.memset(acc16, 0.0)
                eA = e16p = None
                for jt in range(NJT):
                    gs = ib * NJT + jt
                    r = jt % 2
                    p = jt // 2
                    if r == 0:
                        eA = ep.tile([C, 2, A8], F8, tag="ea", name="ea")
                        e16p = ep.tile([C, 2, E16], F16, tag="e16", name="e16")
                    nc.scalar.activation(
                        eA[:, r, :], sp[:, 0:A8],
                        mybir.ActivationFunctionType.Exp,
                        bias=ebias, scale=SCALE / (WSCALE * WSCALE),
                    )
                    # fp16 2^x bits built by the DVE straight into e16p
                    nc.vector.tensor_scalar(
                        e16p[:, r, :].bitcast(I16), sp[:, A8:IB],
                        SCALE / (WSCALE * WSCALE) * A16,
                        EXP_BIAS * A16 + K0,
                        mybir.AluOpType.mult, ADD,
                    )
                    for fn in pre[gs]:
                        fn()
                    if gs + 1 < NSLOT:
                        sp = s_mm(gs + 1)
                    # O: fp16 tail every slot, fp8 DR main on odd slots
                    nc.tensor.matmul(
                        ob[:, A8:IB], uT2[:, p, r, :], e16p[:, r, :],
                        start=(jt == 0), stop=(jt == NJT - 1),
                    )
                    if r == 1:
                        for h in range(2):
                            nc.tensor.matmul(
                                ob[:, h * 416 : (h + 1) * 416],
                                uT2[:, p, :, :],
                                eA[:, :, h * 416 : (h + 1) * 416],
                                start=(p == 0), stop=(p == NPAIR - 1),
                                perf_mode=DRm,
                            )
                    for fn in work[gs]:
                        fn()

                    # colsum for the completed pair, one slot behind so the
                    # DVE starts each slot with the exp trick (which holds
                    # the S PSUM buffer).
                    if r == 1:
                        def cs_fn(eA=eA, e16p=e16p, acc8=acc8, acc16=acc16,
                                  G=GS[ib]):
                            nc.gpsimd.tensor_tensor(
                                acc8[:, :, 0:G], acc8[:, :, 0:G],
                                eA[:, :, 0:G], ADD,
                            )
                            nc.vector.tensor_tensor(
                                acc8[:, :, G:A8], acc8[:, :, G:A8],
                                eA[:, :, G:A8], ADD,
                            )
                            nc.vector.tensor_tensor(acc16, acc16, e16p, ADD)

                        if cs_pending is not None:
                            cs_pending()
                        cs_pending = cs_fn

                # ---- per-ib tail: colsum reduce, reciprocal, normalize ----
                cs_pending()
                cs_pending = None
                accT = fin.tile([C, IB], F16, tag="accT", name="accT")
                nc.vector.tensor_add(accT[:, 0:A8], acc8[:, 0, :], acc8[:, 1, :])
                nc.vector.tensor_add(
                    accT[:, A8:IB], acc16[:, 0, :], acc16[:, 1, :]
                )
                accs_v = accT.rearrange("p (a b) -> p a b", b=8)
                cst = cps.tile([C, 8], F32, tag="conv", name="cst")
                for c8 in range(8):
                    nc.tensor.matmul(
                        cst[:, c8 : c8 + 1], accs_v[:, :, c8], ones,
                        start=True, stop=True,
                    )
                nc.vector.reciprocal(rcp[:, ib, :], cst)
                nc.sync.dma_start(rstage[isl], rcp[:, ib, :])
                rb = fin.tile([C, IB], F32, tag="rb", name="rb")
                nc.sync.dma_start(rb, rstage[isl].partition_broadcast(C))
                if ib < NIB - 1:
                    # mid-run: the DMA bounce and the normalize chunks hide
                    # under the next ib's compute.
                    obe = fin.tile([C, IB], F32, tag="obe", name="obe")
                    nc.vector.tensor_copy(obe, ob)

                    def norm_chunk(chk, ib=ib, obe=obe, rb=rb):
                        csl = slice(ib * IB + chk * 256, ib * IB + (chk + 1) * 256)
                        nt = fin.tile([C, 256], F32, tag="nt", name="nt")
                        nc.vector.tensor_mul(
                            nt, obe[:, chk * 256 : (chk + 1) * 256],
                            rb[:, chk * 256 : (chk + 1) * 256],
                        )
                        ot = fin.tile([C, 256], F16, tag="ot", name="ot")
                        nc.vector.tensor_add(ot, nt, xres[:, csl])
                        nc.sync.dma_start(
                            out_d[:, ib * 16 + chk * 4 : ib * 16 + (chk + 1) * 4, :],
                            ot,
                        )

                    for chk in range(4):
                        work[(ib + 1) * 32 + 4 + 2 * chk].append(
                            lambda chk=chk: norm_chunk(chk)
                        )
                else:
                    # final ib: finely chunked, residual adds alternating
                    # DVE/GPS, stores alternating queues.
                    for chk in range(4):
                        csl = slice(ib * IB + chk * 256, ib * IB + (chk + 1) * 256)
                        nt = fin.tile([C, 256], F32, tag="nt", name="nt")
                        nc.vector.tensor_mul(
                            nt, ob[:, chk * 256 : (chk + 1) * 256],
                            rb[:, chk * 256 : (chk + 1) * 256],
                        )
                        ot = fin.tile([C, 256], F16, tag="ot", name="ot")
                        eng = nc.vector if chk % 2 == 0 else nc.gpsimd
                        eng.tensor_tensor(ot, nt, xres[:, csl], ADD)
                        qeng = nc.sync if chk % 2 == 0 else nc.scalar
                        qeng.dma_start(
                            out_d[:, ib * 16 + chk * 4 : ib * 16 + (chk + 1) * 4, :],
                            ot,
                        )
            dsp.release()
            fin.release()
            ap.release()
            ep.release()
            ops.release()
            sps.release()
            cps.release()

    nc.finalize()
    return nc


def get_nc():
    if "nc" not in _CACHE:
        _CACHE["nc"] = _build_nc()
    return _CACHE["nc"]


def _prep_host_inputs(x, Wq, bq, Wk, bk, Wv, bv, Wo, bo):
    x = np.ascontiguousarray(np.asarray(x, dtype=np.float32))
    Wq = np.asarray(Wq, dtype=np.float32)
    Wk = np.asarray(Wk, dtype=np.float32)
    Wv = np.asarray(Wv, dtype=np.float64)
    Wo2 = np.asarray(Wo, dtype=np.float64).reshape(C, C)
    bq = np.asarray(bq, dtype=np.float32)
    bk = np.asarray(bk, dtype=np.float32)
    bv = np.asarray(bv, dtype=np.float64)
    bo = np.asarray(bo, dtype=np.float64)

    # lhsT layouts: w[c, tap, o] = W[o, c, dy, dx], pre-scaled into fp8 range
    wq = np.ascontiguousarray(Wq.transpose(1, 2, 3, 0).reshape(C, NTAP, C)) * WSCALE
    wk = np.ascontiguousarray(Wk.transpose(1, 2, 3, 0).reshape(C, NTAP, C)) * WSCALE
    Wu = np.einsum("om,mckl->ockl", Wo2, Wv)
    wu = np.ascontiguousarray(Wu.transpose(1, 2, 3, 0).reshape(C, NTAP, C)) * WSCALE
    bu = (Wo2 @ bv + bo).astype(np.float32)

    # img2col in fp8: xcol[b, c, tap, pix] = xpad[b, c, py+dy, px+dx]
    xpad = np.pad(x, ((0, 0), (0, 0), (1, 1), (1, 1)))
    s0, s1, s2, s3 = xpad.strides
    win = np.lib.stride_tricks.as_strided(
        xpad, shape=(B, C, 3, 3, H, W), strides=(s0, s1, s2, s3, s2, s3)
    )
    xcol = np.ascontiguousarray(
        win.reshape(B, C, NTAP, N)
    ).astype(NP8)

    xres = (x.reshape(B, C, N) + bu[None, :, None]).astype(np.float16)

    wks = np.ascontiguousarray(
        np.stack([wk, wq, wu], axis=1)
    ).astype(NP8)  # [C, 3(k,q,u), NTAP, C]
    bqk = np.ascontiguousarray(
        np.stack([bq * WSCALE, bk * WSCALE], axis=1)
    ).astype(np.float32)
    shared = {"wks": wks, "bqk": bqk}
    in_maps = [
        dict(shared, xc=np.ascontiguousarray(xcol[i]), xr=np.ascontiguousarray(xres[i]))
        for i in range(B)
    ]
    return in_maps


def _run(inputs, trace=False):
    from concourse.bass_utils import run_bass_kernel_spmd

    in_maps = _prep_host_inputs(**inputs)
    nc = get_nc()
    res = run_bass_kernel_spmd(nc, in_maps, core_ids=list(range(B)), trace=trace)
    out = np.stack([np.asarray(res.results[i]["out"]) for i in range(B)])
    return out.reshape(B, C, H, W).astype(np.float32), res


def kernel(**inputs) -> np.ndarray:
    out, _ = _run(inputs, trace=False)
    return out


# revision 12
# speedup vs baseline: 1.2225x; 1.2225x over previous
# Trainium2 Bass kernel for AttentionBlock (conv-qkv + spatial softmax attention
# + 1x1 conv out + residual), data-parallel over batch on 8 NeuronCores.
#
# v4: PE-centric rebalance after measuring that DVE and GPSIMD share SBUF
# ports (concurrent GPS ops degrade DVE ~3x, so GPS cannot act as a third
# vector engine).
#   - Convs as before (img2col fp8, DoubleRow, folded u), but ALL conv work
#     (k1-7, q2-7, u4-31) is scheduled inside ib0; ibs 1-3 run conv-free.
#   - E split by i-column: [0, A8) fp8e4 from ACT; [A8, IB) fp16 from the
#     DVE Schraudolph 2^x trick.
#   - O: fp8 region via DoubleRow over jt-pairs; fp16 tail via plain matmuls.
#   - colsum (softmax denominator):
#       ib0    -> DVE elementwise (fp8 at 1x + fp16 at 2x), cst matmuls,
#                 reciprocal, DRAM-bounce broadcast (v2 flow).
#       ib1-3  -> PE ones-DoubleRow matmuls into two PSUM row tiles
#                 (csA/csB) that accumulate over all 16 pairs; the fp16
#                 tail reduces via one [C,E16] matmul from accT16. The two
#                 PSUM banks come from releasing the conv pool after ib0.
#     Denominator rows bounce PSUM->DRAM->[128,8] SBUF for the partition-
#     parallel reciprocal, then DRAM-broadcast as before.
#   - qk conv evictions run on ACT (idle during PE-bound ib0); u evictions
#     and everything else elementwise stays on DVE. GPS only does two tail
#     residual adds (concurrent with an idle DVE).

import numpy as np

try:
    import concourse.bass as bass  # noqa: F401
except ImportError:  # pragma: no cover
    import sys

    sys.path.insert(0, "/opt/trn_rl_repo")

import concourse.bass as bass  # noqa: F401
import concourse.mybir as mybir
import ml_dtypes
from concourse import bacc
from concourse import tile

B = 8
C = 128
H = W = 64
N = H * W  # 4096
NTAP = 9
IB = 1024  # attention i-block (columns per PSUM residency)
NIB = N // IB  # 4
NJT = N // 128  # 32 j-tiles per ib
NPAIR = NJT // 2  # 16 jt-pairs
NSLOT = NIB * NJT  # 128 global slots
WSCALE = 32.0  # fp8 weight pre-scale (host)
SCALE = float(C) ** -0.5
EXP_BIAS = -3.0

F32 = mybir.dt.float32
F16 = mybir.dt.float16
F8 = mybir.dt.float8e4
I16 = mybir.dt.int16
NP8 = ml_dtypes.float8_e4m3
DRm = mybir.MatmulPerfMode.DoubleRow
ADD = mybir.AluOpType.add
Exp = mybir.ActivationFunctionType.Exp
Ident = mybir.ActivationFunctionType.Identity

# E column split. A8 = 688 fp8 cols (ACT), E16 = 336 fp16 cols (DVE trick).
# Chosen so the ones-DR denominators pack exactly into two PSUM banks:
# csA [1,512] covers fp8 cols 0:512; csB holds fp8 cols 512:688 (176 f32)
# plus the fp16-tail sums (336 f32) = 2048B.
A8 = 688
E16 = IB - A8  # 336
CSA = 512  # fp8 cols reduced into csA
CSB = A8 - CSA  # 176, into csB[0:176]; cs16 lands at csB[176:512]
A8H = 352  # DR O-matmul first-half width (16-aligned split of A8)
A16 = 1024.0 * 1.4426950408889634
K0 = 17156.0

_CACHE = {}


def _build_nc():
    nc = bacc.Bacc(None)

    xc_d = nc.dram_tensor("xc", [C, NTAP, N], F8, kind="ExternalInput")
    wks_d = nc.dram_tensor("wks", [C, 3, NTAP, C], F8, kind="ExternalInput")
    bqk_d = nc.dram_tensor("bqk", [C, 2], F32, kind="ExternalInput")
    xr_d = nc.dram_tensor("xr", [C, N], F16, kind="ExternalInput")
    out_d = nc.dram_tensor("out", [C, H, W], F16, kind="ExternalOutput")

    with tile.TileContext(nc) as tc:
        with tc.tile_pool(name="persist", bufs=1) as pp:
            xc = pp.tile([C, NTAP, N], F8)
            xres = pp.tile([C, N], F16)
            qb = pp.tile([C, N], F16)
            kb = pp.tile([C, N], F16)
            uT2 = pp.tile([C, NPAIR, 2, C], F8)  # [j-in-tile, pair, r, c]
            wks = pp.tile([C, 3, NTAP, C], F8)  # packed wk, wq, wu
            bqk = pp.tile([C, 2], F32)
            wk_s, wq_s, wu_s = wks[:, 0], wks[:, 1], wks[:, 2]
            bq_s, bk_s = bqk[:, 0:1], bqk[:, 1:2]
            ebias = pp.tile([C, 1], F32)
            ones = pp.tile([C, 1], F16)
            ones2f = pp.tile([C, 2, 16], F8)
            ones2 = ones2f[:, :, 0:1]  # pair stride 16B satisfies DR step%16
            rcp = pp.tile([C, NIB, 8], F32)
            wrm = pp.tile([C, 512], F16)
            gwrm = pp.tile([C, 8], F16)

            # -------- engine warmups first (before any DMA deps) --------
            nc.vector.memset(wrm, 0.0)
            nc.vector.memset(ebias, EXP_BIAS)
            nc.vector.memset(ones, WSCALE)
            nc.vector.memset(ones2f, WSCALE)
            # GPS: load the tensor_tensor microcode library early
            nc.gpsimd.memset(gwrm, 0.0)
            nc.gpsimd.tensor_tensor(gwrm, gwrm, gwrm, ADD)
            # ACT: pull the exp table load into the DMA window
            awrm = pp.tile([C, 1], F16)
            nc.scalar.activation(awrm, ebias, Exp, bias=0.0, scale=1.0)

            # -------- input DMAs on parallel queues --------
            nc.sync.dma_start(wks, wks_d[:])
            nc.sync.dma_start(bqk, bqk_d[:])
            nc.scalar.dma_start(xc[:, :, 0:512], xc_d[:, :, 0:512])
            nc.scalar.dma_start(xc[:, :, 512:1024], xc_d[:, :, 512:1024])
            nc.scalar.dma_start(xc[:, :, 1024:2048], xc_d[:, :, 1024:2048])
            nc.sync.dma_start(xc[:, :, 2048:3072], xc_d[:, :, 2048:3072])
            nc.sync.dma_start(xc[:, :, 3072:4096], xc_d[:, :, 3072:4096])
            nc.sync.dma_start(xres, xr_d[:])

            sps = tc.alloc_tile_pool(name="sps", bufs=2, space="PSUM")
            ops = tc.alloc_tile_pool(name="ops", bufs=1, space="PSUM")
            ep = tc.alloc_tile_pool(name="ep", bufs=3)
            ap = tc.alloc_tile_pool(name="accp", bufs=2)
            fin = tc.alloc_tile_pool(name="fin", bufs=3)
            dsp = tc.alloc_tile_pool(name="dstage", bufs=1, space="DRAM")
            rstage = dsp.tile([N], F32)
            dstage = dsp.tile([N], F32)  # raw denominator rows (ibs >= 1)
            # conv pool LAST so it can be released after ib0 (stack order)
            # and its two banks re-allocated as the csA/csB denominator pool.
            cps = tc.alloc_tile_pool(name="cps", bufs=2, space="PSUM")
            wps = cps.tile([C, 512], F32, tag="conv", name="wps")
            for _ in range(6):
                nc.tensor.matmul(
                    wps[0:64, :], wrm[:, 0:64], wrm, start=True, stop=True
                )

            # ---------------- conv emission helpers ----------------
            qk_ps = {}

            def qk_pair(ps, w_s, p0, pr):
                for ph in range(2):
                    nc.tensor.matmul(
                        ps[:, ph * 256 : (ph + 1) * 256],
                        w_s[:, 2 * pr : 2 * pr + 2, :],
                        xc[:, 2 * pr : 2 * pr + 2, p0 + ph * 256 : p0 + (ph + 1) * 256],
                        start=False, stop=(pr == 3 and ph == 1), perf_mode=DRm,
                    )

            def qk_group(w_s, b_s, dest, t, g):
                p0 = t * 512
                if g == 0:
                    ps = cps.tile([C, 512], F32, tag="conv", name="cps")
                    qk_ps[id(w_s), t] = ps
                    nc.tensor.matmul(
                        ps, w_s[:, 8, :], xc[:, 8, p0 : p0 + 512],
                        start=True, stop=False,
                    )
                    qk_pair(ps, w_s, p0, 0)
                    return
                ps = qk_ps[id(w_s), t]
                qk_pair(ps, w_s, p0, g)
                if g == 3:
                    del qk_ps[id(w_s), t]
                    # eviction on ACT: out = Identity(ps + bias)
                    nc.scalar.activation(
                        dest[:, p0 : p0 + 512], ps, Ident, bias=b_s, scale=1.0
                    )

            u_ps = {}

            def u_jt(j):
                b = j // 4
                if j % 4 == 0:
                    u_ps[b] = cps.tile([C, 512], F32, tag="conv", name="ups")
                ps = u_ps[b]
                sl = slice((j % 4) * 128, (j % 4 + 1) * 128)
                jp = j * 128
                first = j % 4 == 0
                last = j % 4 == 3
                nc.tensor.matmul(
                    ps[:, sl], xc[:, 8, jp : jp + 128], wu_s[:, 8, :],
                    start=first, stop=False,
                )
                for pr in range(4):
                    nc.tensor.matmul(
                        ps[:, sl],
                        xc[:, 2 * pr : 2 * pr + 2, jp : jp + 128],
                        wu_s[:, 2 * pr : 2 * pr + 2, :],
                        start=False, stop=(last and pr == 3), perf_mode=DRm,
                    )
                if j % 4 == 3:
                    nc.vector.tensor_copy(
                        uT2[:, b * 2 : b * 2 + 2, :, :], u_ps.pop(b)
                    )

            # ---------------- ib0 conv schedule ----------------
            # All of k1-7, q2-7, u4-31 runs inside ib0. Two serial streams
            # share the 2 conv-PSUM bufs: the qk stream paces 2 groups/slot
            # (each tile spans 2 slots); u batches are emitted compressed
            # (4 jts over 2 slots) every 4 slots. Deadlines: kb tile T by
            # slot 4T, uT2 pair 2b by slot 4b, q by slot 32.
            work = [[] for _ in range(NSLOT)]
            prologue = []

            for g in range(4):
                prologue.append(lambda g=g: qk_group(wk_s, bk_s, kb, 0, g))
            for t in range(2):
                for g in range(4):
                    prologue.append(lambda t=t, g=g: qk_group(wq_s, bq_s, qb, t, g))
            for j in range(4):
                prologue.append(lambda j=j: u_jt(j))

            qk_stream = [(wk_s, bk_s, kb, T) for T in range(1, 8)]
            qk_stream += [(wq_s, bq_s, qb, t) for t in range(2, 8)]
            for i, (w, b_, dest, t) in enumerate(qk_stream):
                for g in range(4):
                    s = 1 + 2 * i + g // 2
                    work[s].append(
                        lambda w=w, b_=b_, dest=dest, t=t, g=g: qk_group(w, b_, dest, t, g)
                    )
            for b in range(1, 8):
                for j in range(4 * b, 4 * b + 4):
                    s = max(1, 4 * b - 4) + (j % 4) // 2
                    work[s].append(lambda j=j: u_jt(j))

            # ---------------- S matmul ----------------
            def s_mm(gs):
                ib, jt = gs // NJT, gs % NJT
                sp = sps.tile([C, IB], F32, tag="sp", name="sp")
                for h in range(IB // 512):
                    nc.tensor.matmul(
                        sp[:, h * 512 : (h + 1) * 512],
                        kb[:, jt * 128 : (jt + 1) * 128],
                        qb[:, ib * IB + h * 512 : ib * IB + (h + 1) * 512],
                        start=True, stop=True,
                    )
                return sp

            # ---------------- main loop ----------------
            for fn in prologue:
                fn()

            sp = s_mm(0)
            cs_pending = None
            csp = None
            for ib in range(NIB):
                isl = slice(ib * IB, (ib + 1) * IB)
                ob = ops.tile([C, IB], F32, tag="ob", name="ob")
                acc16 = ap.tile([C, 2, E16], F16, tag="acc16", name="acc16")
                nc.vector.memset(acc16, 0.0)
                if ib == 0:
                    acc8 = ap.tile([C, 2, A8], F16, tag="acc8", name="acc8",
                                   bufs=1)
                    nc.vector.memset(acc8, 0.0)
                else:
                    csA = csp.tile([1, CSA], F32, tag="csA", name="csA",
                                   bufs=1)
                    csB = csp.tile([1, CSA], F32, tag="csB", name="csB",
                                   bufs=1)
                eA = e16p = None
                for jt in range(NJT):
                    gs = ib * NJT + jt
                    r = jt % 2
                    p = jt // 2
                    if r == 0:
                        eA = ep.tile([C, 2, A8], F8, tag="ea", name="ea")
                        e16p = ep.tile([C, 2, E16], F16, tag="e16", name="e16")
                    nc.scalar.activation(
                        eA[:, r, :], sp[:, 0:A8], Exp,
                        bias=ebias, scale=SCALE / (WSCALE * WSCALE),
                    )
                    nc.vector.tensor_scalar(
                        e16p[:, r, :].bitcast(I16), sp[:, A8:IB],
                        SCALE / (WSCALE * WSCALE) * A16,
                        EXP_BIAS * A16 + K0,
                        mybir.AluOpType.mult, ADD,
                    )
                    if gs + 1 < NSLOT:
                        sp = s_mm(gs + 1)
                    # O: fp16 tail every slot, fp8 DR main on odd slots
                    nc.tensor.matmul(
                        ob[:, A8:IB], uT2[:, p, r, :], e16p[:, r, :],
                        start=(jt == 0), stop=(jt == NJT - 1),
                    )
                    if r == 1:
                        for c0, c1 in ((0, A8H), (A8H, A8)):
                            nc.tensor.matmul(
                                ob[:, c0:c1],
                                uT2[:, p, :, :],
                                eA[:, :, c0:c1],
                                start=(p == 0), stop=(p == NPAIR - 1),
                                perf_mode=DRm,
                            )
                        if ib > 0:
                            # denominator partials on the PE: ones-DR over
                            # the same eA stream
                            nc.tensor.matmul(
                                csA, ones2, eA[:, :, 0:CSA],
                                start=(p == 0), stop=(p == NPAIR - 1),
                                perf_mode=DRm,
                            )
                            nc.tensor.matmul(
                                csB[:, 0:CSB], ones2, eA[:, :, CSA:A8],
                                start=(p == 0), stop=(p == NPAIR - 1),
                                perf_mode=DRm,
                            )
                    for fn in work[gs]:
                        fn()

                    # deferred-by-one-slot DVE colsum (fp16 tail everywhere,
                    # plus the fp8 region in ib0 only)
                    if r == 1:
                        def cs_fn(eA=eA, e16p=e16p, acc16=acc16, ib=ib):
                            nc.vector.tensor_tensor(acc16, acc16, e16p, ADD)
                            if ib == 0:
                                nc.vector.tensor_tensor(acc8, acc8, eA, ADD)

                        if cs_pending is not None:
                            cs_pending()
                        cs_pending = cs_fn

                # ---- per-ib tail: denominators, reciprocal, normalize ----
                cs_pending()
                cs_pending = None
                accT16 = fin.tile([C, E16], F16, tag="accT16", name="accT16")
                nc.vector.tensor_add(accT16, acc16[:, 0, :], acc16[:, 1, :])
                if ib == 0:
                    accT = fin.tile([C, IB], F16, tag="accT", name="accT")
                    nc.vector.tensor_add(
                        accT[:, 0:A8], acc8[:, 0, :], acc8[:, 1, :]
                    )
                    nc.vector.tensor_copy(accT[:, A8:IB], accT16)
                    accs_v = accT.rearrange("p (a b) -> p a b", b=8)
                    cst = cps.tile([C, 8], F32, tag="conv", name="cst")
                    for c8 in range(8):
                        nc.tensor.matmul(
                            cst[:, c8 : c8 + 1], accs_v[:, :, c8], ones,
                            start=True, stop=True,
                        )
                    nc.vector.reciprocal(rcp[:, ib, :], cst)
                else:
                    # fp16-tail reduction joins the PSUM denominator rows
                    nc.tensor.matmul(
                        csB[:, CSB : CSB + E16], ones, accT16,
                        start=True, stop=True,
                    )
                    csv = fin.tile([1, IB], F32, tag="csv", name="csv")
                    nc.vector.tensor_copy(csv[:, 0:CSA], csA)
                    nc.vector.tensor_copy(csv[:, CSA:IB], csB)
                    nc.sync.dma_start(dstage[isl], csv)
                    den = fin.tile([C, 8], F32, tag="den", name="den")
                    nc.sync.dma_start(den, dstage[isl])
                    nc.vector.reciprocal(rcp[:, ib, :], den)
                nc.sync.dma_start(rstage[isl], rcp[:, ib, :])
                rb = fin.tile([C, IB], F32, tag="rb", name="rb")
                nc.sync.dma_start(rb, rstage[isl].partition_broadcast(C))
                if ib == 0:
                    # conv PSUM banks become the csA/csB denominator banks
                    cps.release()
                    csp = tc.alloc_tile_pool(name="csp", bufs=1, space="PSUM")
                if ib < NIB - 1:
                    obe = fin.tile([C, IB], F32, tag="obe", name="obe")
                    nc.scalar.activation(obe[:, 0:512], ob[:, 0:512],
                                         Ident, bias=0.0, scale=1.0)
                    nc.vector.tensor_copy(obe[:, 512:IB], ob[:, 512:IB])

                    def norm_chunk(chk, ib=ib, obe=obe, rb=rb):
                        csl = slice(ib * IB + chk * 256, ib * IB + (chk + 1) * 256)
                        nt = fin.tile([C, 256], F32, tag="nt", name="nt")
                        nc.vector.tensor_mul(
                            nt, obe[:, chk * 256 : (chk + 1) * 256],
                            rb[:, chk * 256 : (chk + 1) * 256],
                        )
                        ot = fin.tile([C, 256], F16, tag="ot", name="ot")
                        nc.vector.tensor_add(ot, nt, xres[:, csl])
                        nc.sync.dma_start(
                            out_d[:, ib * 16 + chk * 4 : ib * 16 + (chk + 1) * 4, :],
                            ot,
                        )

                    for chk in range(4):
                        work[(ib + 1) * 32 + 4 + 2 * chk].append(
                            lambda chk=chk: norm_chunk(chk)
                        )
                else:
                    for chk in range(4):
                        csl = slice(ib * IB + chk * 256, ib * IB + (chk + 1) * 256)
                        nt = fin.tile([C, 256], F32, tag="nt", name="nt")
                        nc.vector.tensor_mul(
                            nt, ob[:, chk * 256 : (chk + 1) * 256],
                            rb[:, chk * 256 : (chk + 1) * 256],
                        )
                        ot = fin.tile([C, 256], F16, tag="ot", name="ot")
                        eng = nc.vector if chk % 2 == 0 else nc.gpsimd
                        eng.tensor_tensor(ot, nt, xres[:, csl], ADD)
                        qeng = nc.sync if chk % 2 == 0 else nc.scalar
                        qeng.dma_start(
                            out_d[:, ib * 16 + chk * 4 : ib * 16 + (chk + 1) * 4, :],
                            ot,
                        )
            csp.release()
            dsp.release()
            fin.release()
            ap.release()
            ep.release()
            ops.release()
            sps.release()

    nc.finalize()
    return nc


def get_nc():
    if "nc" not in _CACHE:
        _CACHE["nc"] = _build_nc()
    return _CACHE["nc"]


def _prep_host_inputs(x, Wq, bq, Wk, bk, Wv, bv, Wo, bo):
    x = np.ascontiguousarray(np.asarray(x, dtype=np.float32))
    Wq = np.asarray(Wq, dtype=np.float32)
    Wk = np.asarray(Wk, dtype=np.float32)
    Wv = np.asarray(Wv, dtype=np.float64)
    Wo2 = np.asarray(Wo, dtype=np.float64).reshape(C, C)
    bq = np.asarray(bq, dtype=np.float32)
    bk = np.asarray(bk, dtype=np.float32)
    bv = np.asarray(bv, dtype=np.float64)
    bo = np.asarray(bo, dtype=np.float64)

    wq = np.ascontiguousarray(Wq.transpose(1, 2, 3, 0).reshape(C, NTAP, C)) * WSCALE
    wk = np.ascontiguousarray(Wk.transpose(1, 2, 3, 0).reshape(C, NTAP, C)) * WSCALE
    Wu = np.einsum("om,mckl->ockl", Wo2, Wv)
    wu = np.ascontiguousarray(Wu.transpose(1, 2, 3, 0).reshape(C, NTAP, C)) * WSCALE
    bu = (Wo2 @ bv + bo).astype(np.float32)

    xpad = np.pad(x, ((0, 0), (0, 0), (1, 1), (1, 1)))
    s0, s1, s2, s3 = xpad.strides
    win = np.lib.stride_tricks.as_strided(
        xpad, shape=(B, C, 3, 3, H, W), strides=(s0, s1, s2, s3, s2, s3)
    )
    xcol = np.ascontiguousarray(win.reshape(B, C, NTAP, N)).astype(NP8)

    xres = (x.reshape(B, C, N) + bu[None, :, None]).astype(np.float16)

    wks = np.ascontiguousarray(np.stack([wk, wq, wu], axis=1)).astype(NP8)
    bqk = np.ascontiguousarray(
        np.stack([bq * WSCALE, bk * WSCALE], axis=1)
    ).astype(np.float32)
    shared = {"wks": wks, "bqk": bqk}
    in_maps = [
        dict(shared, xc=np.ascontiguousarray(xcol[i]), xr=np.ascontiguousarray(xres[i]))
        for i in range(B)
    ]
    return in_maps


def _run(inputs, trace=False):
    from concourse.bass_utils import run_bass_kernel_spmd

    in_maps = _prep_host_inputs(**inputs)
    nc = get_nc()
    res = run_bass_kernel_spmd(nc, in_maps, core_ids=list(range(B)), trace=trace)
    out = np.stack([np.asarray(res.results[i]["out"]) for i in range(B)])
    return out.reshape(B, C, H, W).astype(np.float32), res


def kernel(**inputs) -> np.ndarray:
    out, _ = _run(inputs, trace=False)
    return out


# revision 15
# speedup vs baseline: 1.2637x; 1.0337x over previous
# Trainium2 Bass kernel for AttentionBlock (conv-qkv + spatial softmax attention
# + 1x1 conv out + residual), data-parallel over batch on 8 NeuronCores.
#
# v4: PE-centric rebalance after measuring that DVE and GPSIMD share SBUF
# ports (concurrent GPS ops degrade DVE ~3x, so GPS cannot act as a third
# vector engine).
#   - Convs as before (img2col fp8, DoubleRow, folded u), but ALL conv work
#     (k1-7, q2-7, u4-31) is scheduled inside ib0; ibs 1-3 run conv-free.
#   - E split by i-column: [0, A8) fp8e4 from ACT; [A8, IB) fp16 from the
#     DVE Schraudolph 2^x trick.
#   - O: fp8 region via DoubleRow over jt-pairs; fp16 tail via plain matmuls.
#   - colsum (softmax denominator):
#       ib0    -> DVE elementwise (fp8 at 1x + fp16 at 2x), cst matmuls,
#                 reciprocal, DRAM-bounce broadcast (v2 flow).
#       ib1-3  -> PE ones-DoubleRow matmuls into two PSUM row tiles
#                 (csA/csB) that accumulate over all 16 pairs; the fp16
#                 tail reduces via one [C,E16] matmul from accT16. The two
#                 PSUM banks come from releasing the conv pool after ib0.
#     Denominator rows bounce PSUM->DRAM->[128,8] SBUF for the partition-
#     parallel reciprocal, then DRAM-broadcast as before.
#   - qk conv evictions run on ACT (idle during PE-bound ib0); u evictions
#     and everything else elementwise stays on DVE. GPS only does two tail
#     residual adds (concurrent with an idle DVE).

import numpy as np

try:
    import concourse.bass as bass  # noqa: F401
except ImportError:  # pragma: no cover
    import sys

    sys.path.insert(0, "/opt/trn_rl_repo")

import concourse.bass as bass  # noqa: F401
import concourse.mybir as mybir
import ml_dtypes
from concourse import bacc
from concourse import tile

B = 8
C = 128
H = W = 64
N = H * W  # 4096
NTAP = 9
IB = 1024  # attention i-block (columns per PSUM residency)
NIB = N // IB  # 4
NJT = N // 128  # 32 j-tiles per ib
NPAIR = NJT // 2  # 16 jt-pairs
NSLOT = NIB * NJT  # 128 global slots
WSCALE = 32.0  # fp8 weight pre-scale (host)
SCALE = float(C) ** -0.5
EXP_BIAS = -3.0

F32 = mybir.dt.float32
F16 = mybir.dt.float16
F8 = mybir.dt.float8e4
I16 = mybir.dt.int16
NP8 = ml_dtypes.float8_e4m3
DRm = mybir.MatmulPerfMode.DoubleRow
ADD = mybir.AluOpType.add
Exp = mybir.ActivationFunctionType.Exp
Ident = mybir.ActivationFunctionType.Identity

# E column split. A8 = 688 fp8 cols (ACT), E16 = 336 fp16 cols (DVE trick).
# Chosen so the ones-DR denominators pack exactly into two PSUM banks:
# csA [1,512] covers fp8 cols 0:512; csB holds fp8 cols 512:688 (176 f32)
# plus the fp16-tail sums (336 f32) = 2048B.
A8 = 848
E16 = IB - A8  # 176
CSA = 512  # fp8 cols reduced into csA
CSB = A8 - CSA  # 336, into csB[0:336]; cs16 lands at csB[336:512]
A8H = 512  # DR O-matmul first-half width (PSUM-bank-aligned split of A8)
A16 = 1024.0 * 1.4426950408889634
K0 = 17156.0

_CACHE = {}


def _build_nc():
    nc = bacc.Bacc(None)

    xc_d = nc.dram_tensor("xc", [C, NTAP, N], F8, kind="ExternalInput")
    wks_d = nc.dram_tensor("wks", [C, 3, NTAP, C], F8, kind="ExternalInput")
    bqk_d = nc.dram_tensor("bqk", [C, 2], F32, kind="ExternalInput")
    xr_d = nc.dram_tensor("xr", [C, N], F16, kind="ExternalInput")
    out_d = nc.dram_tensor("out", [C, H, W], F16, kind="ExternalOutput")

    with tile.TileContext(nc) as tc:
        with tc.tile_pool(name="persist", bufs=1) as pp:
            xc = pp.tile([C, NTAP, N], F8)
            xres = pp.tile([C, N], F16)
            qb = pp.tile([C, N], F16)
            kb = pp.tile([C, N], F16)
            uT2 = pp.tile([C, NPAIR, 2, C], F8)  # [j-in-tile, pair, r, c]
            wks = pp.tile([C, 3, NTAP, C], F8)  # packed wk, wq, wu
            bqk = pp.tile([C, 2], F32)
            wk_s, wq_s, wu_s = wks[:, 0], wks[:, 1], wks[:, 2]
            bq_s, bk_s = bqk[:, 0:1], bqk[:, 1:2]
            ebias = pp.tile([C, 1], F32)
            ones = pp.tile([C, 1], F16)
            ones2f = pp.tile([C, 2, 16], F8)
            ones2 = ones2f[:, :, 0:1]  # pair stride 16B satisfies DR step%16
            rcp = pp.tile([C, NIB, 8], F32)
            wrm = pp.tile([C, 512], F16)
            gwrm = pp.tile([C, 8], F16)

            # -------- engine warmups first (before any DMA deps) --------
            nc.vector.memset(wrm, 0.0)
            nc.vector.memset(ebias, EXP_BIAS)
            nc.vector.memset(ones, WSCALE)
            nc.vector.memset(ones2f, WSCALE)
            # GPS: load the tensor_tensor microcode library early
            nc.gpsimd.memset(gwrm, 0.0)
            nc.gpsimd.tensor_tensor(gwrm, gwrm, gwrm, ADD)
            # ACT: pull the exp table load into the DMA window
            awrm = pp.tile([C, 1], F16)
            nc.scalar.activation(awrm, ebias, Exp, bias=0.0, scale=1.0)

            # -------- input DMAs on parallel queues --------
            nc.sync.dma_start(wks, wks_d[:])
            nc.sync.dma_start(bqk, bqk_d[:])
            nc.scalar.dma_start(xc[:, :, 0:512], xc_d[:, :, 0:512])
            nc.scalar.dma_start(xc[:, :, 512:1024], xc_d[:, :, 512:1024])
            nc.scalar.dma_start(xc[:, :, 1024:2048], xc_d[:, :, 1024:2048])
            nc.sync.dma_start(xc[:, :, 2048:3072], xc_d[:, :, 2048:3072])
            nc.sync.dma_start(xc[:, :, 3072:4096], xc_d[:, :, 3072:4096])
            nc.sync.dma_start(xres, xr_d[:])

            sps = tc.alloc_tile_pool(name="sps", bufs=2, space="PSUM")
            ops = tc.alloc_tile_pool(name="ops", bufs=1, space="PSUM")
            ep = tc.alloc_tile_pool(name="ep", bufs=3)
            ap = tc.alloc_tile_pool(name="accp", bufs=2)
            fin = tc.alloc_tile_pool(name="fin", bufs=3)
            dsp = tc.alloc_tile_pool(name="dstage", bufs=1, space="DRAM")
            rstage = dsp.tile([N], F32)
            dstage = dsp.tile([N], F32)  # raw denominator rows (ibs >= 1)
            # conv pool LAST so it can be released after ib0 (stack order)
            # and its two banks re-allocated as the csA/csB denominator pool.
            cps = tc.alloc_tile_pool(name="cps", bufs=2, space="PSUM")
            wps = cps.tile([C, 512], F32, tag="conv", name="wps")
            for _ in range(6):
                nc.tensor.matmul(
                    wps[0:64, :], wrm[:, 0:64], wrm, start=True, stop=True
                )

            # ---------------- conv emission helpers ----------------
            qk_ps = {}

            def qk_pair(ps, w_s, p0, pr):
                for ph in range(2):
                    nc.tensor.matmul(
                        ps[:, ph * 256 : (ph + 1) * 256],
                        w_s[:, 2 * pr : 2 * pr + 2, :],
                        xc[:, 2 * pr : 2 * pr + 2, p0 + ph * 256 : p0 + (ph + 1) * 256],
                        start=False, stop=(pr == 3 and ph == 1), perf_mode=DRm,
                    )

            def qk_group(w_s, b_s, dest, t, g):
                p0 = t * 512
                if g == 0:
                    ps = cps.tile([C, 512], F32, tag="conv", name="cps")
                    qk_ps[id(w_s), t] = ps
                    nc.tensor.matmul(
                        ps, w_s[:, 8, :], xc[:, 8, p0 : p0 + 512],
                        start=True, stop=False,
                    )
                    qk_pair(ps, w_s, p0, 0)
                    return
                ps = qk_ps[id(w_s), t]
                qk_pair(ps, w_s, p0, g)
                if g == 3:
                    del qk_ps[id(w_s), t]
                    # eviction on ACT: out = Identity(ps + bias)
                    nc.scalar.activation(
                        dest[:, p0 : p0 + 512], ps, Ident, bias=b_s, scale=1.0
                    )

            u_ps = {}

            def u_jt(j):
                b = j // 4
                if j % 4 == 0:
                    u_ps[b] = cps.tile([C, 512], F32, tag="conv", name="ups")
                ps = u_ps[b]
                sl = slice((j % 4) * 128, (j % 4 + 1) * 128)
                jp = j * 128
                first = j % 4 == 0
                last = j % 4 == 3
                nc.tensor.matmul(
                    ps[:, sl], xc[:, 8, jp : jp + 128], wu_s[:, 8, :],
                    start=first, stop=False,
                )
                for pr in range(4):
                    nc.tensor.matmul(
                        ps[:, sl],
                        xc[:, 2 * pr : 2 * pr + 2, jp : jp + 128],
                        wu_s[:, 2 * pr : 2 * pr + 2, :],
                        start=False, stop=(last and pr == 3), perf_mode=DRm,
                    )
                if j % 4 == 3:
                    nc.vector.tensor_copy(
                        uT2[:, b * 2 : b * 2 + 2, :, :], u_ps.pop(b)
                    )

            # ---------------- ib0 conv schedule ----------------
            # All of k1-7, q2-7, u4-31 runs inside ib0. Two serial streams
            # share the 2 conv-PSUM bufs: the qk stream paces 2 groups/slot
            # (each tile spans 2 slots); u batches are emitted compressed
            # (4 jts over 2 slots) every 4 slots. Deadlines: kb tile T by
            # slot 4T, uT2 pair 2b by slot 4b, q by slot 32.
            work = [[] for _ in range(NSLOT)]
            prologue = []

            for g in range(4):
                prologue.append(lambda g=g: qk_group(wk_s, bk_s, kb, 0, g))
            for t in range(2):
                for g in range(4):
                    prologue.append(lambda t=t, g=g: qk_group(wq_s, bq_s, qb, t, g))
            for j in range(4):
                prologue.append(lambda j=j: u_jt(j))

            qk_stream = [(wk_s, bk_s, kb, T) for T in range(1, 8)]
            qk_stream += [(wq_s, bq_s, qb, t) for t in range(2, 8)]
            for i, (w, b_, dest, t) in enumerate(qk_stream):
                for g in range(4):
                    s = 1 + 2 * i + g // 2
                    work[s].append(
                        lambda w=w, b_=b_, dest=dest, t=t, g=g: qk_group(w, b_, dest, t, g)
                    )
            for b in range(1, 8):
                for j in range(4 * b, 4 * b + 4):
                    s = max(1, 4 * b - 4) + (j % 4) // 2
                    work[s].append(lambda j=j: u_jt(j))

            # ---------------- S matmul ----------------
            def s_mm(gs):
                ib, jt = gs // NJT, gs % NJT
                sp = sps.tile([C, IB], F32, tag="sp", name="sp")
                for h in range(IB // 512):
                    nc.tensor.matmul(
                        sp[:, h * 512 : (h + 1) * 512],
                        kb[:, jt * 128 : (jt + 1) * 128],
                        qb[:, ib * IB + h * 512 : ib * IB + (h + 1) * 512],
                        start=True, stop=True,
                    )
                return sp

            # ---------------- main loop ----------------
            for fn in prologue:
                fn()

            sp = s_mm(0)
            cs_pending = None
            csp = None
            for ib in range(NIB):
                isl = slice(ib * IB, (ib + 1) * IB)
                ob = ops.tile([C, IB], F32, tag="ob", name="ob")
                acc16 = ap.tile([C, 2, E16], F16, tag="acc16", name="acc16")
                nc.vector.memset(acc16, 0.0)
                if ib == 0:
                    acc8 = ap.tile([C, 2, A8], F16, tag="acc8", name="acc8",
                                   bufs=1)
                    nc.vector.memset(acc8, 0.0)
                else:
                    csA = csp.tile([1, CSA], F32, tag="csA", name="csA",
                                   bufs=1)
                    csB = csp.tile([1, CSA], F32, tag="csB", name="csB",
                                   bufs=1)
                eA = e16p = None
                for jt in range(NJT):
                    gs = ib * NJT + jt
                    r = jt % 2
                    p = jt // 2
                    if r == 0:
                        eA = ep.tile([C, 2, A8], F8, tag="ea", name="ea")
                        e16p = ep.tile([C, 2, E16], F16, tag="e16", name="e16")
                    nc.scalar.activation(
                        eA[:, r, :], sp[:, 0:A8], Exp,
                        bias=ebias, scale=SCALE / (WSCALE * WSCALE),
                    )
                    nc.vector.tensor_scalar(
                        e16p[:, r, :].bitcast(I16), sp[:, A8:IB],
                        SCALE / (WSCALE * WSCALE) * A16,
                        EXP_BIAS * A16 + K0,
                        mybir.AluOpType.mult, ADD,
                    )
                    if gs + 1 < NSLOT:
                        sp = s_mm(gs + 1)
                    # O matmuls grouped by stationary operand: DR pair, then
                    # the ones-DR denominators, then the fp16 tail.
                    if r == 1:
                        for c0, c1 in ((0, A8H), (A8H, A8)):
                            nc.tensor.matmul(
                                ob[:, c0:c1],
                                uT2[:, p, :, :],
                                eA[:, :, c0:c1],
                                start=(p == 0), stop=(p == NPAIR - 1),
                                perf_mode=DRm,
                            )
                        if ib > 0:
                            # denominator partials on the PE over the same
                            # eA stream; csA split in two to stay off the
                            # 1024-free-dim slow path
                            for c0, c1 in ((0, 256), (256, CSA)):
                                nc.tensor.matmul(
                                    csA[:, c0:c1], ones2, eA[:, :, c0:c1],
                                    start=(p == 0), stop=(p == NPAIR - 1),
                                    perf_mode=DRm,
                                )
                            nc.tensor.matmul(
                                csB[:, 0:CSB], ones2, eA[:, :, CSA:A8],
                                start=(p == 0), stop=(p == NPAIR - 1),
                                perf_mode=DRm,
                            )
                    nc.tensor.matmul(
                        ob[:, A8:IB], uT2[:, p, r, :], e16p[:, r, :],
                        start=(jt == 0), stop=(jt == NJT - 1),
                    )
                    for fn in work[gs]:
                        fn()

                    # deferred-by-one-slot DVE colsum (fp16 tail everywhere,
                    # plus the fp8 region in ib0 only)
                    if r == 1:
                        def cs_fn(eA=eA, e16p=e16p, acc16=acc16, ib=ib):
                            nc.vector.tensor_tensor(acc16, acc16, e16p, ADD)
                            if ib == 0:
                                nc.vector.tensor_tensor(acc8, acc8, eA, ADD)

                        if cs_pending is not None:
                            cs_pending()
                        cs_pending = cs_fn

                # ---- per-ib tail: denominators, reciprocal, normalize ----
                cs_pending()
                cs_pending = None
                accT16 = fin.tile([C, E16], F16, tag="accT16", name="accT16")
                nc.vector.tensor_add(accT16, acc16[:, 0, :], acc16[:, 1, :])
                if ib == 0:
                    accT = fin.tile([C, IB], F16, tag="accT", name="accT")
                    nc.vector.tensor_add(
                        accT[:, 0:A8], acc8[:, 0, :], acc8[:, 1, :]
                    )
                    nc.vector.tensor_copy(accT[:, A8:IB], accT16)
                    accs_v = accT.rearrange("p (a b) -> p a b", b=8)
                    cst = cps.tile([C, 8], F32, tag="conv", name="cst")
                    for c8 in range(8):
                        nc.tensor.matmul(
                            cst[:, c8 : c8 + 1], accs_v[:, :, c8], ones,
                            start=True, stop=True,
                        )
                    nc.vector.reciprocal(rcp[:, ib, :], cst)
                else:
                    # fp16-tail reduction joins the PSUM denominator rows
                    nc.tensor.matmul(
                        csB[:, CSB : CSB + E16], ones, accT16,
                        start=True, stop=True,
                    )
                    csv = fin.tile([1, IB], F32, tag="csv", name="csv")
                    nc.vector.tensor_copy(csv[:, 0:CSA], csA)
                    nc.vector.tensor_copy(csv[:, CSA:IB], csB)
                    nc.sync.dma_start(dstage[isl], csv)
                    den = fin.tile([C, 8], F32, tag="den", name="den")
                    nc.sync.dma_start(den, dstage[isl])
                    nc.vector.reciprocal(rcp[:, ib, :], den)
                nc.sync.dma_start(rstage[isl], rcp[:, ib, :])
                rb = fin.tile([C, IB], F32, tag="rb", name="rb")
                nc.sync.dma_start(rb, rstage[isl].partition_broadcast(C))
                if ib == 0:
                    # conv PSUM banks become the csA/csB denominator banks
                    cps.release()
                    csp = tc.alloc_tile_pool(name="csp", bufs=1, space="PSUM")
                if ib < NIB - 1:
                    obe = fin.tile([C, IB], F32, tag="obe", name="obe")
                    nc.vector.tensor_copy(obe, ob)

                    def norm_chunk(chk, ib=ib, obe=obe, rb=rb):
                        csl = slice(ib * IB + chk * 256, ib * IB + (chk + 1) * 256)
                        nt = fin.tile([C, 256], F32, tag="nt", name="nt")
                        nc.vector.tensor_mul(
                            nt, obe[:, chk * 256 : (chk + 1) * 256],
                            rb[:, chk * 256 : (chk + 1) * 256],
                        )
                        ot = fin.tile([C, 256], F16, tag="ot", name="ot")
                        nc.vector.tensor_add(ot, nt, xres[:, csl])
                        nc.sync.dma_start(
                            out_d[:, ib * 16 + chk * 4 : ib * 16 + (chk + 1) * 4, :],
                            ot,
                        )

                    for chk in range(4):
                        work[(ib + 1) * 32 + 4 + 2 * chk].append(
                            lambda chk=chk: norm_chunk(chk)
                        )
                else:
                    for chk in range(4):
                        csl = slice(ib * IB + chk * 256, ib * IB + (chk + 1) * 256)
                        nt = fin.tile([C, 256], F32, tag="nt", name="nt")
                        nc.vector.tensor_mul(
                            nt, ob[:, chk * 256 : (chk + 1) * 256],
                            rb[:, chk * 256 : (chk + 1) * 256],
                        )
                        ot = fin.tile([C, 256], F16, tag="ot", name="ot")
                        eng = nc.vector if chk % 2 == 0 else nc.gpsimd
                        eng.tensor_tensor(ot, nt, xres[:, csl], ADD)
                        qeng = nc.sync if chk % 2 == 0 else nc.scalar
                        qeng.dma_start(
                            out_d[:, ib * 16 + chk * 4 : ib * 16 + (chk + 1) * 4, :],
                            ot,
                        )
            csp.release()
            dsp.release()
            fin.release()
            ap.release()
            ep.release()
            ops.release()
            sps.release()

    nc.finalize()
    return nc


def get_nc():
    if "nc" not in _CACHE:
        _CACHE["nc"] = _build_nc()
    return _CACHE["nc"]


def _prep_host_inputs(x, Wq, bq, Wk, bk, Wv, bv, Wo, bo):
    x = np.ascontiguousarray(np.asarray(x, dtype=np.float32))
    Wq = np.asarray(Wq, dtype=np.float32)
    Wk = np.asarray(Wk, dtype=np.float32)
    Wv = np.asarray(Wv, dtype=np.float64)
    Wo2 = np.asarray(Wo, dtype=np.float64).reshape(C, C)
    bq = np.asarray(bq, dtype=np.float32)
    bk = np.asarray(bk, dtype=np.float32)
    bv = np.asarray(bv, dtype=np.float64)
    bo = np.asarray(bo, dtype=np.float64)

    wq = np.ascontiguousarray(Wq.transpose(1, 2, 3, 0).reshape(C, NTAP, C)) * WSCALE
    wk = np.ascontiguousarray(Wk.transpose(1, 2, 3, 0).reshape(C, NTAP, C)) * WSCALE
    Wu = np.einsum("om,mckl->ockl", Wo2, Wv)
    wu = np.ascontiguousarray(Wu.transpose(1, 2, 3, 0).reshape(C, NTAP, C)) * WSCALE
    bu = (Wo2 @ bv + bo).astype(np.float32)

    xpad = np.pad(x, ((0, 0), (0, 0), (1, 1), (1, 1)))
    s0, s1, s2, s3 = xpad.strides
    win = np.lib.stride_tricks.as_strided(
        xpad, shape=(B, C, 3, 3, H, W), strides=(s0, s1, s2, s3, s2, s3)
    )
    xcol = np.ascontiguousarray(win.reshape(B, C, NTAP, N)).astype(NP8)

    xres = (x.reshape(B, C, N) + bu[None, :, None]).astype(np.float16)

    wks = np.ascontiguousarray(np.stack([wk, wq, wu], axis=1)).astype(NP8)
    bqk = np.ascontiguousarray(
        np.stack([bq * WSCALE, bk * WSCALE], axis=1)
    ).astype(np.float32)
    shared = {"wks": wks, "bqk": bqk}
    in_maps = [
        dict(shared, xc=np.ascontiguousarray(xcol[i]), xr=np.ascontiguousarray(xres[i]))
        for i in range(B)
    ]
    return in_maps


def _run(inputs, trace=False):
    from concourse.bass_utils import run_bass_kernel_spmd

    in_maps = _prep_host_inputs(**inputs)
    nc = get_nc()
    res = run_bass_kernel_spmd(nc, in_maps, core_ids=list(range(B)), trace=trace)
    out = np.stack([np.asarray(res.results[i]["out"]) for i in range(B)])
    return out.reshape(B, C, H, W).astype(np.float32), res


def kernel(**inputs) -> np.ndarray:
    out, _ = _run(inputs, trace=False)
    return out


# revision 22
# speedup vs baseline: 1.3164x; 1.0417x over previous
# Trainium2 Bass kernel for AttentionBlock (conv-qkv + spatial softmax attention
# + 1x1 conv out + residual), data-parallel over batch on 8 NeuronCores.
#
# v4: PE-centric rebalance after measuring that DVE and GPSIMD share SBUF
# ports (concurrent GPS ops degrade DVE ~3x, so GPS cannot act as a third
# vector engine).
#   - Convs as before (img2col fp8, DoubleRow, folded u), but ALL conv work
#     (k1-7, q2-7, u4-31) is scheduled inside ib0; ibs 1-3 run conv-free.
#   - E split by i-column: [0, A8) fp8e4 from ACT; [A8, IB) fp16 from the
#     DVE Schraudolph 2^x trick.
#   - O: fp8 region via DoubleRow over jt-pairs; fp16 tail via plain matmuls.
#   - colsum (softmax denominator):
#       ib0    -> DVE elementwise (fp8 at 1x + fp16 at 2x), cst matmuls,
#                 reciprocal, DRAM-bounce broadcast (v2 flow).
#       ib1-3  -> PE ones-DoubleRow matmuls into two PSUM row tiles
#                 (csA/csB) that accumulate over all 16 pairs; the fp16
#                 tail reduces via one [C,E16] matmul from accT16. The two
#                 PSUM banks come from releasing the conv pool after ib0.
#     Denominator rows bounce PSUM->DRAM->[128,8] SBUF for the partition-
#     parallel reciprocal, then DRAM-broadcast as before.
#   - qk conv evictions run on ACT (idle during PE-bound ib0); u evictions
#     and everything else elementwise stays on DVE. GPS only does two tail
#     residual adds (concurrent with an idle DVE).

import numpy as np

try:
    import concourse.bass as bass  # noqa: F401
except ImportError:  # pragma: no cover
    import sys

    sys.path.insert(0, "/opt/trn_rl_repo")

import concourse.bass as bass  # noqa: F401
import concourse.mybir as mybir
import ml_dtypes
from concourse import bacc
from concourse import tile

B = 8
C = 128
H = W = 64
N = H * W  # 4096
NTAP = 9
IB = 1024  # attention i-block (columns per PSUM residency)
NIB = N // IB  # 4
NJT = N // 128  # 32 j-tiles per ib
NPAIR = NJT // 2  # 16 jt-pairs
NSLOT = NIB * NJT  # 128 global slots
WSCALE = 32.0  # fp8 weight pre-scale (host)
SCALE = float(C) ** -0.5
EXP_BIAS = -3.0

F32 = mybir.dt.float32
F16 = mybir.dt.float16
F8 = mybir.dt.float8e4
I16 = mybir.dt.int16
NP8 = ml_dtypes.float8_e4m3
DRm = mybir.MatmulPerfMode.DoubleRow
ADD = mybir.AluOpType.add
Exp = mybir.ActivationFunctionType.Exp
Ident = mybir.ActivationFunctionType.Identity

# E column split. A8 = 688 fp8 cols (ACT), E16 = 336 fp16 cols (DVE trick).
# Chosen so the ones-DR denominators pack exactly into two PSUM banks:
# csA [1,512] covers fp8 cols 0:512; csB holds fp8 cols 512:688 (176 f32)
# plus the fp16-tail sums (336 f32) = 2048B.
A8 = 848
E16 = IB - A8  # 176
CSA = 512  # fp8 cols reduced into csA
CSB = A8 - CSA  # 336, into csB[0:336]; cs16 lands at csB[336:512]
A8H = 512  # DR O-matmul first-half width (PSUM-bank-aligned split of A8)
A16 = 1024.0 * 1.4426950408889634
K0 = 17156.0

_CACHE = {}


def _build_nc():
    nc = bacc.Bacc(None)

    xc_d = nc.dram_tensor("xc", [C, NTAP, N], F8, kind="ExternalInput")
    wks_d = nc.dram_tensor("wks", [C, 3, NTAP, C], F8, kind="ExternalInput")
    bqk_d = nc.dram_tensor("bqk", [C, 2], F32, kind="ExternalInput")
    xr_d = nc.dram_tensor("xr", [C, N], F16, kind="ExternalInput")
    out_d = nc.dram_tensor("out", [C, H, W], F16, kind="ExternalOutput")

    with tile.TileContext(nc) as tc:
        with tc.tile_pool(name="persist", bufs=1) as pp:
            xc = pp.tile([C, NTAP, N], F8)
            xres = pp.tile([C, N], F16)
            qb = pp.tile([C, N], F16)
            kb = pp.tile([C, N], F16)
            uT2 = pp.tile([C, NPAIR, 2, C], F8)  # [j-in-tile, pair, r, c]
            wks = pp.tile([C, 3, NTAP, C], F8)  # packed wk, wq, wu
            bqk = pp.tile([C, 2], F32)
            wk_s, wq_s, wu_s = wks[:, 0], wks[:, 1], wks[:, 2]
            bq_s, bk_s = bqk[:, 0:1], bqk[:, 1:2]
            ebias = pp.tile([C, 1], F32)
            ones = pp.tile([C, 1], F16)
            ones2f = pp.tile([C, 2, 16], F8)
            ones2 = ones2f[:, :, 0:1]  # pair stride 16B satisfies DR step%16
            rcp = pp.tile([C, NIB, 8], F32)
            wrm = pp.tile([C, 512], F16)
            gwrm = pp.tile([C, 8], F16)

            # -------- engine warmups first (before any DMA deps) --------
            nc.vector.memset(wrm, 0.0)
            nc.vector.memset(ebias, EXP_BIAS)
            nc.vector.memset(ones, WSCALE)
            nc.vector.memset(ones2f, WSCALE)
            # GPS: load the tensor_tensor microcode library early
            nc.gpsimd.memset(gwrm, 0.0)
            nc.gpsimd.tensor_tensor(gwrm, gwrm, gwrm, ADD)
            # ACT: pull the exp table load into the DMA window
            awrm = pp.tile([C, 1], F16)
            nc.scalar.activation(awrm, ebias, Exp, bias=0.0, scale=1.0)

            # -------- input DMAs on parallel queues --------
            nc.sync.dma_start(wks, wks_d[:])
            nc.sync.dma_start(bqk, bqk_d[:])
            nc.scalar.dma_start(xc[:, :, 0:512], xc_d[:, :, 0:512])
            nc.scalar.dma_start(xc[:, :, 512:1024], xc_d[:, :, 512:1024])
            nc.scalar.dma_start(xc[:, :, 1024:2048], xc_d[:, :, 1024:2048])
            nc.sync.dma_start(xc[:, :, 2048:3072], xc_d[:, :, 2048:3072])
            nc.sync.dma_start(xc[:, :, 3072:4096], xc_d[:, :, 3072:4096])
            nc.sync.dma_start(xres, xr_d[:])

            sps = tc.alloc_tile_pool(name="sps", bufs=2, space="PSUM")
            ops = tc.alloc_tile_pool(name="ops", bufs=1, space="PSUM")
            ep = tc.alloc_tile_pool(name="ep", bufs=3)
            ap = tc.alloc_tile_pool(name="accp", bufs=2)
            fin = tc.alloc_tile_pool(name="fin", bufs=3)
            dsp = tc.alloc_tile_pool(name="dstage", bufs=1, space="DRAM")
            rstage = dsp.tile([N], F32)
            dstage = dsp.tile([N], F32)  # raw denominator rows (ibs >= 1)
            # conv pool LAST so it can be released after ib0 (stack order)
            # and its two banks re-allocated as the csA/csB denominator pool.
            cps = tc.alloc_tile_pool(name="cps", bufs=2, space="PSUM")
            wps = cps.tile([C, 512], F32, tag="conv", name="wps")
            for _ in range(6):
                nc.tensor.matmul(
                    wps[0:64, :], wrm[:, 0:64], wrm, start=True, stop=True
                )

            # ---------------- conv emission helpers ----------------
            qk_ps = {}

            def qk_pair(ps, w_s, p0, pr):
                for ph in range(2):
                    nc.tensor.matmul(
                        ps[:, ph * 256 : (ph + 1) * 256],
                        w_s[:, 2 * pr : 2 * pr + 2, :],
                        xc[:, 2 * pr : 2 * pr + 2, p0 + ph * 256 : p0 + (ph + 1) * 256],
                        start=False, stop=(pr == 3 and ph == 1), perf_mode=DRm,
                    )

            def qk_group(w_s, b_s, dest, t, g):
                p0 = t * 512
                if g == 0:
                    ps = cps.tile([C, 512], F32, tag="conv", name="cps")
                    qk_ps[id(w_s), t] = ps
                    nc.tensor.matmul(
                        ps, w_s[:, 8, :], xc[:, 8, p0 : p0 + 512],
                        start=True, stop=False,
                    )
                    qk_pair(ps, w_s, p0, 0)
                    return
                ps = qk_ps[id(w_s), t]
                qk_pair(ps, w_s, p0, g)
                if g == 3:
                    del qk_ps[id(w_s), t]
                    # eviction on ACT: out = Identity(ps + bias)
                    nc.scalar.activation(
                        dest[:, p0 : p0 + 512], ps, Ident, bias=b_s, scale=1.0
                    )

            u_ps = {}

            def u_jt(j):
                b = j // 4
                if j % 4 == 0:
                    u_ps[b] = cps.tile([C, 512], F32, tag="conv", name="ups")
                ps = u_ps[b]
                sl = slice((j % 4) * 128, (j % 4 + 1) * 128)
                jp = j * 128
                first = j % 4 == 0
                last = j % 4 == 3
                nc.tensor.matmul(
                    ps[:, sl], xc[:, 8, jp : jp + 128], wu_s[:, 8, :],
                    start=first, stop=False,
                )
                for pr in range(4):
                    nc.tensor.matmul(
                        ps[:, sl],
                        xc[:, 2 * pr : 2 * pr + 2, jp : jp + 128],
                        wu_s[:, 2 * pr : 2 * pr + 2, :],
                        start=False, stop=(last and pr == 3), perf_mode=DRm,
                    )
                if j % 4 == 3:
                    nc.vector.tensor_copy(
                        uT2[:, b * 2 : b * 2 + 2, :, :], u_ps.pop(b)
                    )

            # ---------------- ib0 conv schedule ----------------
            # All of k1-7, q2-7, u4-31 runs inside ib0. Two serial streams
            # share the 2 conv-PSUM bufs: the qk stream paces 2 groups/slot
            # (each tile spans 2 slots); u batches are emitted compressed
            # (4 jts over 2 slots) every 4 slots. Deadlines: kb tile T by
            # slot 4T, uT2 pair 2b by slot 4b, q by slot 32.
            work = [[] for _ in range(NSLOT)]
            prologue = []

            for g in range(4):
                prologue.append(lambda g=g: qk_group(wk_s, bk_s, kb, 0, g))
            for t in range(2):
                for g in range(4):
                    prologue.append(lambda t=t, g=g: qk_group(wq_s, bq_s, qb, t, g))
            for j in range(4):
                prologue.append(lambda j=j: u_jt(j))

            qk_stream = [(wk_s, bk_s, kb, T) for T in range(1, 8)]
            qk_stream += [(wq_s, bq_s, qb, t) for t in range(2, 8)]
            for i, (w, b_, dest, t) in enumerate(qk_stream):
                for g in range(4):
                    s = 1 + 2 * i + g // 2
                    work[s].append(
                        lambda w=w, b_=b_, dest=dest, t=t, g=g: qk_group(w, b_, dest, t, g)
                    )
            for b in range(1, 8):
                for j in range(4 * b, 4 * b + 4):
                    s = max(1, 4 * b - 4) + (j % 4) // 2
                    work[s].append(lambda j=j: u_jt(j))

            # ---------------- S matmul ----------------
            def s_mm(gs):
                ib, jt = gs // NJT, gs % NJT
                sp = sps.tile([C, IB], F32, tag="sp", name="sp")
                for h in range(IB // 512):
                    nc.tensor.matmul(
                        sp[:, h * 512 : (h + 1) * 512],
                        kb[:, jt * 128 : (jt + 1) * 128],
                        qb[:, ib * IB + h * 512 : ib * IB + (h + 1) * 512],
                        start=True, stop=True,
                    )
                return sp

            # ---------------- main loop ----------------
            for fn in prologue:
                fn()

            sp = s_mm(0)
            cs_pending = None
            o_pending = None
            csp = None
            for ib in range(NIB):
                isl = slice(ib * IB, (ib + 1) * IB)
                ob = ops.tile([C, IB], F32, tag="ob", name="ob")
                acc16 = ap.tile([C, 2, E16], F16, tag="acc16", name="acc16")
                if ib == 0:
                    acc8 = ap.tile([C, 2, A8], F16, tag="acc8", name="acc8",
                                   bufs=1)
                else:
                    csA = csp.tile([1, CSA], F32, tag="csA", name="csA",
                                   bufs=1)
                    csB = csp.tile([1, CSA], F32, tag="csB", name="csB",
                                   bufs=1)
                eA = e16p = None
                for jt in range(NJT):
                    gs = ib * NJT + jt
                    r = jt % 2
                    p = jt // 2
                    if r == 0:
                        eA = ep.tile([C, 2, A8], F8, tag="ea", name="ea")
                        e16p = ep.tile([C, 2, E16], F16, tag="e16", name="e16")
                    nc.scalar.activation(
                        eA[:, r, :], sp[:, 0:A8], Exp,
                        bias=ebias, scale=SCALE / (WSCALE * WSCALE),
                    )
                    nc.vector.tensor_scalar(
                        e16p[:, r, :].bitcast(I16), sp[:, A8:IB],
                        SCALE / (WSCALE * WSCALE) * A16,
                        EXP_BIAS * A16 + K0,
                        mybir.AluOpType.mult, ADD,
                    )
                    if gs + 1 < NSLOT:
                        sp = s_mm(gs + 1)
                    # The O/denominator matmuls for slot s are emitted at
                    # slot s+1 so they sit BEHIND the next S in the PE FIFO
                    # and never make the PE block on the current exp.
                    if o_pending is not None:
                        o_pending()

                    def o_fn(eA=eA, e16p=e16p, ob=ob, jt=jt, r=r, p=p,
                             ib=ib, csA=(None if ib == 0 else csA),
                             csB=(None if ib == 0 else csB)):
                        if r == 1:
                            for c0, c1 in ((0, A8H), (A8H, A8)):
                                nc.tensor.matmul(
                                    ob[:, c0:c1],
                                    uT2[:, p, :, :],
                                    eA[:, :, c0:c1],
                                    start=(p == 0), stop=(p == NPAIR - 1),
                                    perf_mode=DRm,
                                )
                            if ib > 0:
                                # csA split in two to stay off the
                                # 1024-free-dim slow path
                                for c0, c1 in ((0, 256), (256, CSA)):
                                    nc.tensor.matmul(
                                        csA[:, c0:c1], ones2,
                                        eA[:, :, c0:c1],
                                        start=(p == 0),
                                        stop=(p == NPAIR - 1),
                                        perf_mode=DRm,
                                    )
                                nc.tensor.matmul(
                                    csB[:, 0:CSB], ones2, eA[:, :, CSA:A8],
                                    start=(p == 0), stop=(p == NPAIR - 1),
                                    perf_mode=DRm,
                                )
                        nc.tensor.matmul(
                            ob[:, A8:IB], uT2[:, p, r, :], e16p[:, r, :],
                            start=(jt == 0), stop=(jt == NJT - 1),
                        )

                    o_pending = o_fn
                    for fn in work[gs]:
                        fn()

                    # deferred-by-one-slot DVE colsum (fp16 tail everywhere,
                    # plus the fp8 region in ib0 only)
                    if r == 1:
                        def cs_fn(eA=eA, e16p=e16p, acc16=acc16, ib=ib, p=p):
                            if p == 0:
                                nc.vector.tensor_copy(acc16, e16p)
                                if ib == 0:
                                    nc.vector.tensor_copy(acc8, eA)
                            else:
                                nc.vector.tensor_tensor(acc16, acc16, e16p, ADD)
                                if ib == 0:
                                    nc.vector.tensor_tensor(acc8, acc8, eA, ADD)

                        if cs_pending is not None:
                            cs_pending()
                        cs_pending = cs_fn

                # ---- per-ib tail: denominators, reciprocal, normalize ----
                o_pending()
                o_pending = None
                cs_pending()
                cs_pending = None
                accT16 = fin.tile([C, E16], F16, tag="accT16", name="accT16")
                nc.vector.tensor_add(accT16, acc16[:, 0, :], acc16[:, 1, :])
                if ib == 0:
                    accT = fin.tile([C, IB], F16, tag="accT", name="accT")
                    nc.vector.tensor_add(
                        accT[:, 0:A8], acc8[:, 0, :], acc8[:, 1, :]
                    )
                    nc.vector.tensor_copy(accT[:, A8:IB], accT16)
                    accs_v = accT.rearrange("p (a b) -> p a b", b=8)
                    cst = cps.tile([C, 8], F32, tag="conv", name="cst")
                    for c8 in range(8):
                        nc.tensor.matmul(
                            cst[:, c8 : c8 + 1], accs_v[:, :, c8], ones,
                            start=True, stop=True,
                        )
                    nc.vector.reciprocal(rcp[:, ib, :], cst)
                else:
                    # fp16-tail reduction joins the PSUM denominator rows
                    nc.tensor.matmul(
                        csB[:, CSB : CSB + E16], ones, accT16,
                        start=True, stop=True,
                    )
                    csv = fin.tile([1, IB], F32, tag="csv", name="csv")
                    nc.vector.tensor_copy(csv[:, 0:CSA], csA)
                    nc.vector.tensor_copy(csv[:, CSA:IB], csB)
                    nc.sync.dma_start(dstage[isl], csv)
                    den = fin.tile([C, 8], F32, tag="den", name="den")
                    nc.sync.dma_start(den, dstage[isl])
                    nc.vector.reciprocal(rcp[:, ib, :], den)
                nc.sync.dma_start(rstage[isl], rcp[:, ib, :])
                rb = fin.tile([C, IB], F32, tag="rb", name="rb")
                nc.sync.dma_start(rb, rstage[isl].partition_broadcast(C))
                if ib == 0:
                    # conv PSUM banks become the csA/csB denominator banks
                    cps.release()
                    csp = tc.alloc_tile_pool(name="csp", bufs=1, space="PSUM")
                if ib < NIB - 1:
                    # obe on ACT: the exp stream has a natural gap at the
                    # boundary while the DVE is busiest here
                    obe = fin.tile([C, IB], F32, tag="obe", name="obe")
                    nc.scalar.activation(obe, ob, Ident, bias=0.0, scale=1.0)

                    def norm_chunk(chk, ib=ib, obe=obe, rb=rb):
                        csl = slice(ib * IB + chk * 256, ib * IB + (chk + 1) * 256)
                        nt = fin.tile([C, 256], F32, tag="nt", name="nt")
                        nc.vector.tensor_mul(
                            nt, obe[:, chk * 256 : (chk + 1) * 256],
                            rb[:, chk * 256 : (chk + 1) * 256],
                        )
                        ot = fin.tile([C, 256], F16, tag="ot", name="ot")
                        nc.vector.tensor_add(ot, nt, xres[:, csl])
                        nc.sync.dma_start(
                            out_d[:, ib * 16 + chk * 4 : ib * 16 + (chk + 1) * 4, :],
                            ot,
                        )

                    for chk in range(4):
                        work[(ib + 1) * 32 + 10 + 4 * chk].append(
                            lambda chk=chk: norm_chunk(chk)
                        )
                else:
                    for chk in range(4):
                        csl = slice(ib * IB + chk * 256, ib * IB + (chk + 1) * 256)
                        nt = fin.tile([C, 256], F32, tag="nt", name="nt")
                        nc.vector.tensor_mul(
                            nt, ob[:, chk * 256 : (chk + 1) * 256],
                            rb[:, chk * 256 : (chk + 1) * 256],
                        )
                        ot = fin.tile([C, 256], F16, tag="ot", name="ot")
                        eng = nc.vector if chk % 2 == 0 else nc.gpsimd
                        eng.tensor_tensor(ot, nt, xres[:, csl], ADD)
                        qeng = nc.sync if chk % 2 == 0 else nc.scalar
                        qeng.dma_start(
                            out_d[:, ib * 16 + chk * 4 : ib * 16 + (chk + 1) * 4, :],
                            ot,
                        )
            csp.release()
            dsp.release()
            fin.release()
            ap.release()
            ep.release()
            ops.release()
            sps.release()

    nc.finalize()
    return nc


def get_nc():
    if "nc" not in _CACHE:
        _CACHE["nc"] = _build_nc()
    return _CACHE["nc"]


def _prep_host_inputs(x, Wq, bq, Wk, bk, Wv, bv, Wo, bo):
    x = np.ascontiguousarray(np.asarray(x, dtype=np.float32))
    Wq = np.asarray(Wq, dtype=np.float32)
    Wk = np.asarray(Wk, dtype=np.float32)
    Wv = np.asarray(Wv, dtype=np.float64)
    Wo2 = np.asarray(Wo, dtype=np.float64).reshape(C, C)
    bq = np.asarray(bq, dtype=np.float32)
    bk = np.asarray(bk, dtype=np.float32)
    bv = np.asarray(bv, dtype=np.float64)
    bo = np.asarray(bo, dtype=np.float64)

    wq = np.ascontiguousarray(Wq.transpose(1, 2, 3, 0).reshape(C, NTAP, C)) * WSCALE
    wk = np.ascontiguousarray(Wk.transpose(1, 2, 3, 0).reshape(C, NTAP, C)) * WSCALE
    Wu = np.einsum("om,mckl->ockl", Wo2, Wv)
    wu = np.ascontiguousarray(Wu.transpose(1, 2, 3, 0).reshape(C, NTAP, C)) * WSCALE
    bu = (Wo2 @ bv + bo).astype(np.float32)

    xpad = np.pad(x, ((0, 0), (0, 0), (1, 1), (1, 1)))
    s0, s1, s2, s3 = xpad.strides
    win = np.lib.stride_tricks.as_strided(
        xpad, shape=(B, C, 3, 3, H, W), strides=(s0, s1, s2, s3, s2, s3)
    )
    xcol = np.ascontiguousarray(win.reshape(B, C, NTAP, N)).astype(NP8)

    xres = (x.reshape(B, C, N) + bu[None, :, None]).astype(np.float16)

    wks = np.ascontiguousarray(np.stack([wk, wq, wu], axis=1)).astype(NP8)
    bqk = np.ascontiguousarray(
        np.stack([bq * WSCALE, bk * WSCALE], axis=1)
    ).astype(np.float32)
    shared = {"wks": wks, "bqk": bqk}
    in_maps = [
        dict(shared, xc=np.ascontiguousarray(xcol[i]), xr=np.ascontiguousarray(xres[i]))
        for i in range(B)
    ]
    return in_maps


def _run(inputs, trace=False):
    from concourse.bass_utils import run_bass_kernel_spmd

    in_maps = _prep_host_inputs(**inputs)
    nc = get_nc()
    res = run_bass_kernel_spmd(nc, in_maps, core_ids=list(range(B)), trace=trace)
    out = np.stack([np.asarray(res.results[i]["out"]) for i in range(B)])
    return out.reshape(B, C, H, W).astype(np.float32), res


def kernel(**inputs) -> np.ndarray:
    out, _ = _run(inputs, trace=False)
    return out
